# revision 23
# baseline (speedup 1.0000x reference)
"""Bass/Tile TRN2 kernel for nn_ExpressionAttentionLayer.

Math per batch b (B=8, G=2048, D=64):
    K_fused = concat([K_gene, K_expr], -1) @ WK_w.T + WK_b      # (G, D)
    Q_fused = concat([Q_gene, Q_expr], -1) @ WQ_w.T + WQ_b      # (G, D)
    A       = softmax(Q_fused @ K_fused.T / sqrt(D), axis=-1)
    out     = (A * M) @ V_expr                                   # (G, D)

Sharding: data-parallel over batch; core i handles batch i (B == n_cores == 8).
No collectives.

Per-core dataflow (v3):
  - All Q/K loads use the "(p s)" partition-contiguous DRAM layout (1KB+
    descriptors).  PE transpose-mode runs directly on fp32 (1 pass on
    cayman), so no pre-casts.
  - K side feeds both HWDGE rings first (kfT gates every logits matmul);
    its transposed chunks scatter to stride-16 column views to restore
    natural k order (k must line up with M's columns).
  - Q side keeps the chunk order: loop iteration qt computes the 128
    logical q rows {p*16 + qt}.  The M tile and the output tile for
    iteration qt use row-permuted DRAM views (M stays 8KB/partition
    descriptors), so no on-chip fixup is needed and iteration qt only
    depends on Q chunk qt -> the loop starts ~7us in.
  - M streams in as bf16 via SWDGE cast-DMA half-tiles on the gpsimd
    queue (its own ring; 16 MB of the ~19.4 MB total HBM read traffic).
  - Per q-tile, per 1024-col half h: logits psum(fp32) = Q_tile.T @
    K_fusedT (bf16); exp on ScalarE -> bf16 ex with fp32 row-sum accum
    (scale=1/sqrt(D) folded in; |logits| <~ 7 so no max-subtraction
    needed); em = ex * M_bf16 on VectorE (all-bf16 -> 2x DVE mode);
    PE-transpose em (bf16 psum); single DVE 2x copy to [k, q] sbuf
    tiles; accumulate out over k-tiles on PE; apply the softmax
    reciprocal on ScalarE while copying out of PSUM; DMA out on sync.
"""

from contextlib import ExitStack

import numpy as np

import concourse.bass as bass
import concourse.tile as tile
from concourse import bacc, mybir
from concourse.bass_utils import run_bass_kernel_spmd
from concourse.masks import make_identity

B, G, D = 8, 2048, 64
P = 128
NT = G // P  # 16 tiles of 128 rows
F32 = mybir.dt.float32
BF16 = mybir.dt.bfloat16
AF = mybir.ActivationFunctionType

N_CORES = 8


def _emit(ctx: ExitStack, tc: tile.TileContext, io: dict):
    nc = tc.nc

    singles = ctx.enter_context(tc.tile_pool(name="singles", bufs=1))
    ld = ctx.enter_context(tc.tile_pool(name="ld", bufs=4))

    # PSUM pools (8 banks total: ps_l 2x2 + ps_t 2x1 + ps_o 2x1 = 8)
    ps_l = ctx.enter_context(tc.tile_pool(name="ps_l", bufs=2, space="PSUM"))
    ps_t = ctx.enter_context(tc.tile_pool(name="ps_t", bufs=2, space="PSUM"))
    ps_o = ctx.enter_context(tc.tile_pool(name="ps_o", bufs=2, space="PSUM"))

    identity = singles.tile([P, P], F32)
    make_identity(nc, identity[:])
    identity_bf = singles.tile([P, P], BF16)
    nc.vector.tensor_copy(identity_bf[:], identity[:])

    # ---- M-tile SWDGE cast-DMA lookahead queue (bf16 in SBUF) ----
    # Row-permuted view: iteration qt covers logical q rows {p*16 + qt},
    # matching the Q-side chunk order (see module docstring).  8KB
    # contiguous per partition per half -> line-rate descriptors.
    mpool = ctx.enter_context(tc.tile_pool(name="mpool", bufs=6))
    m_r = io["M"].rearrange("(p s) k -> s p k", s=NT)
    mts = {}

    def issue_m(qt, poke=None):
        if qt < NT:
            mt = mpool.tile([P, G], BF16, tag="m", name="m")
            if poke is not None:
                # WAW dep: the DMA overwrites this poked region, so it cannot
                # start before the poke, which in turn reads the barrier
                # dummy -> M tile qt is truly held behind the input loads.
                # On DVE (not Pool) so the Pool queue can't self-deadlock.
                nc.vector.tensor_copy(mt[0:1, 0:D], poke[0:1, :])
            for h in range(2):
                nc.gpsimd.dma_start(
                    mt[:, h * 1024 : (h + 1) * 1024],
                    m_r[qt, :, h * 1024 : (h + 1) * 1024],
                )
            mts[qt] = mt

    # Only M0 streams alongside the prologue input loads; M1+ are held
    # behind a gpsimd-queue barrier (emitted after the input loads below)
    # so the 16 MB M stream can't starve the latency-critical input phase.
    issue_m(0)

    # ---- HAM warmup bridge: ~4.5us of junk matmuls so the PE has no idle
    # window between kernel start and the first input transposes (a PE-idle
    # HAM window re-throttles the clock to 1.2 GHz and it can stick there).
    junk = singles.tile([P, 512], BF16, tag="junk")
    nc.vector.memset(junk[:], 0.0)
    for _ in range(16):
        psw = ps_o.tile([P, 512], F32, tag="ps_o", name="ps_warm")
        nc.tensor.matmul(psw[:], identity_bf[:], junk[:], start=True, stop=True)

    # ---- weights: WK_w/WQ_w are [D, 2D]; natural load, then PE-transpose the
    # two [64, 64] halves (base partition 0) and cast to bf16 lhsT tiles.
    wk_nat = singles.tile([D, 2 * D], F32, tag="wk_nat")
    wq_nat = singles.tile([D, 2 * D], F32, tag="wq_nat")
    nc.sync.dma_start(wk_nat[:], io["WK_w"][:, :])
    nc.scalar.dma_start(wq_nat[:], io["WQ_w"][:, :])
    wk_gTb = singles.tile([D, D], BF16, tag="wk_gTb")
    wk_eTb = singles.tile([D, D], BF16, tag="wk_eTb")
    wq_gTb = singles.tile([D, D], BF16, tag="wq_gTb")
    wq_eTb = singles.tile([D, D], BF16, tag="wq_eTb")
    for nat, dsts in ((wk_nat, (wk_gTb, wk_eTb)), (wq_nat, (wq_gTb, wq_eTb))):
        for h, dst in enumerate(dsts):
            psw = ps_o.tile([P, 512], F32, tag="ps_o", name="ps_w")
            nc.tensor.transpose(
                psw[:D, :D], nat[:, h * D : (h + 1) * D], identity[:D, :D]
            )
            nc.vector.tensor_copy(dst[:], psw[:D, :D])
    wkb = singles.tile([D, 1], F32, tag="wkb")
    wqb = singles.tile([D, 1], F32, tag="wqb")
    nc.sync.dma_start(wkb[:], io["WK_b"][:, None])
    nc.scalar.dma_start(wqb[:], io["WQ_b"][:, None])

    # ---- input loads: "(p s)" partition-contiguous layout, quartered.
    # K side split across BOTH HWDGE rings (it gates everything); Q side
    # follows on both rings; V last (first needed by the AV stage).
    bigs = {}
    for src_name, eng in (
        ("K_gene", nc.sync),
        ("K_expr", nc.scalar),
        ("Q_gene", nc.sync),
        ("Q_expr", nc.scalar),
    ):
        big = ld.tile([P, NT, D], F32, tag=f"ld_{src_name}", name=f"ld_{src_name}")
        r = io[src_name].rearrange("(p s) d -> p s d", s=NT)
        for hq in range(2):
            eng.dma_start(big[:, 8 * hq : 8 * hq + 8, :], r[:, 8 * hq : 8 * hq + 8, :])
        bigs[src_name] = big

    # bf16 pre-casts so the PE transposes are 1-pass bf16; K side on DVE,
    # Q side on ACT — both near-idle in the prologue.  Half-MAJOR emission
    # order: each engine queue is in-order, so a tensor's h1 cast (late
    # data) must not sit in front of another tensor's h0 cast (early data).
    bigs_bf = {}
    for src_name in ("K_gene", "K_expr", "Q_gene", "Q_expr"):
        bigs_bf[src_name] = ld.tile(
            [P, NT, D], BF16, tag=f"ldb_{src_name}", name=f"ldb_{src_name}"
        )
    for hq in range(2):
        for src_name in ("K_gene", "K_expr", "Q_gene", "Q_expr"):
            sl = (slice(None), slice(8 * hq, 8 * hq + 8), slice(None))
            if src_name.startswith("K"):
                nc.vector.tensor_copy(bigs_bf[src_name][sl], bigs[src_name][sl])
            else:
                nc.scalar.copy(bigs_bf[src_name][sl], bigs[src_name][sl])

    # ---- V two-stage load: the natural [128, kt, 64] layout has 256B HBM
    # descriptors (~13x below line rate), so load the contiguous "(p s)"
    # layout at line rate into a staging tile, then redistribute on-chip
    # with 16 SBUF->SBUF chunk DMAs (no small-descriptor penalty on-chip).
    v_ps = ld.tile([P, NT, D], F32, tag="v_ps", name="v_ps")
    nc.scalar.dma_start(v_ps[:], io["V_expr"].rearrange("(p s) d -> p s d", s=NT))
    v_sb = singles.tile([P, NT, D], F32, tag="v")
    for kt in range(NT):
        nc.sync.dma_start(v_sb[:, kt, :], v_ps[8 * kt : 8 * kt + 8, :, :])
    v_bf = singles.tile([P, NT, D], BF16, tag="v_bf")

    # ---- input barrier: a dummy read of the last input halves, chained
    # into a poke-write on each held-back M tile (WAW dep with its DMA), so
    # M1+ genuinely cannot stream until the input loads have landed.
    barrier_dummy = singles.tile([1, D], F32, tag="barrier_dummy")
    nc.gpsimd.tensor_copy(barrier_dummy[:], bigs["Q_gene"][0:1, NT - 1, :])
    nc.gpsimd.tensor_copy(barrier_dummy[:], bigs["Q_expr"][0:1, NT - 1, :])
    for _qt in range(1, 5):
        issue_m(_qt, poke=barrier_dummy)

    # ---- transpose K/Q gene+expr into bf16 [D, G] (d on partitions) ----
    # Chunk s of the "(p s)" load transposes to columns {p*16 + s}.  K side
    # scatters through a stride-16 view to restore natural k order; Q side
    # keeps chunk order (the q permutation is absorbed by the M/out views).
    kgT = singles.tile([D, G], BF16, tag="kgT")
    keT = singles.tile([D, G], BF16, tag="keT")
    qgT = singles.tile([D, G], BF16, tag="qgT")
    qeT = singles.tile([D, G], BF16, tag="qeT")
    kfT = singles.tile([D, G], BF16, tag="kfT")
    qfT = singles.tile([D, G], BF16, tag="qfT")

    def emit_transposes(side, gT, eT, j):
        # Both sides store CHUNK order: chunk s -> columns [s*128,(s+1)*128),
        # holding g = p*16 + s at position p.  All copies are contiguous.
        for c, dstT, ceng in ((0, gT, nc.vector), (1, eT, nc.scalar)):
            big = bigs_bf[f"{side}_gene" if c == 0 else f"{side}_expr"]
            ps = ps_t.tile([P, 8 * P], BF16, tag="ps_t", name="ps_tr")[:D, : 4 * P]
            for i in range(4):
                s = 4 * j + i
                nc.tensor.transpose(
                    ps[:, i * P : (i + 1) * P], big[:, s, :], identity_bf[:]
                )
            dst = dstT[:, j * 512 : (j + 1) * 512].rearrange("d (i p) -> d i p", i=4)
            src = ps[:].rearrange("d (i p) -> d i p", i=4)
            if c == 0:
                ceng.tensor_copy(dst, src)
            else:
                ceng.copy(dst, src)

    def emit_proj(gT, eT, wgT, weT, b_sb, fT, j, permute):
        # K side: the chunk->natural column reorder rides the projection's
        # moving-operand AP (the PE AP walker streams any column order at
        # line rate): natural column g <-> chunk position (g%16)*128 + g//16.
        if permute:
            rg = gT[:].rearrange("d (s p) -> d p s", s=NT)[:, j * 32 : (j + 1) * 32, :]
            re = eT[:].rearrange("d (s p) -> d p s", s=NT)[:, j * 32 : (j + 1) * 32, :]
        else:
            rg = gT[:, j * 512 : (j + 1) * 512]
            re = eT[:, j * 512 : (j + 1) * 512]
        psj = ps_o.tile([P, 512], F32, tag="ps_o", name="ps_pj")[:D, :]
        nc.tensor.matmul(psj[:], wgT[:], rg, start=True, stop=False)
        nc.tensor.matmul(psj[:], weT[:], re, start=False, stop=True)
        if permute:
            # bias-add + bf16 cast on DVE (ACT is busier at this point)
            nc.vector.tensor_scalar_add(
                fT[:, j * 512 : (j + 1) * 512], psj[:], b_sb[:, 0:1]
            )
        else:
            nc.scalar.activation(
                fT[:, j * 512 : (j + 1) * 512], psj[:], AF.Identity, bias=b_sb[:, 0:1]
            )

    for j in range(4):
        emit_transposes("K", kgT, keT, j)
    for j in range(4):
        # K projections need all K chunks (each natural 512-block reads
        # one position from every chunk); they gate every logits matmul,
        # so they come before the Q-side PE work.
        emit_proj(kgT, keT, wk_gTb, wk_eTb, wkb, kfT, j, permute=True)
    # Q block 0 only — blocks 1-3 are emitted inside the first loop
    # iterations (the loop is DMA-bound, PE has slack), so the loop starts
    # as soon as block 0 is projected.  Q keeps chunk order end-to-end
    # (M/out row views absorb it): block j only needs chunks 4j..4j+3.
    emit_transposes("Q", qgT, qeT, 0)
    emit_proj(qgT, qeT, wq_gTb, wq_eTb, wqb, qfT, 0, permute=False)

    def emit_q_block(j):
        emit_transposes("Q", qgT, qeT, j)
        emit_proj(qgT, qeT, wq_gTb, wq_eTb, wqb, qfT, j, permute=False)

    # V cast lands here in DVE program order (V arrives ~mid-prologue)
    nc.vector.tensor_copy(v_bf[:], v_sb[:])

    # ---- main attention loop (fully per-q-tile pipelined) ----
    epool = ctx.enter_context(tc.tile_pool(name="epool", bufs=2))
    empool = ctx.enter_context(tc.tile_pool(name="empool", bufs=2))
    tpool = ctx.enter_context(tc.tile_pool(name="tpool", bufs=2))
    opool = ctx.enter_context(tc.tile_pool(name="opool", bufs=2))
    rspool = ctx.enter_context(tc.tile_pool(name="rspool", bufs=4))

    out_r = io["out"].rearrange("(p s) d -> s p d", s=NT)
    scale = 1.0 / np.sqrt(np.float32(D))

    # Per-qt state carried one step so the AV matmuls of qt-1 are emitted
    # between qt's logits and qt's transposes — PE chews on them while the
    # ScalarE/VectorE stages of qt run, instead of stalling at a group
    # barrier.
    pending = None  # (qt, emt, recip)

    def emit_av(pend):
        qt_p, emt_p, recip_p = pend
        # out[q, d] += expM^T_chunk.T @ V  (lhsT=emt chunk: 128 bf16 cols -> FWL)
        pso = ps_o.tile([P, 512], F32, tag="ps_o", name="ps_av")[:, :D]
        for kt in range(NT):
            nc.tensor.matmul(
                pso[:],
                emt_p[:, kt, :],
                v_bf[:, kt, :],
                start=(kt == 0),
                stop=(kt == NT - 1),
            )
        ob = opool.tile([P, D], F32, tag="ob")
        # apply softmax denominator while copying out of PSUM
        nc.scalar.activation(ob[:], pso[:], AF.Copy, bias=0.0, scale=recip_p[:, 0:1])
        nc.sync.dma_start(out_r[qt_p], ob[:])

    for qt in range(NT):
        mt = mts.pop(qt)
        issue_m(qt + 5)

        ex = epool.tile([P, G], BF16, tag="ex")
        em = empool.tile([P, G], BF16, tag="em")
        emt = tpool.tile([P, NT, P], BF16, tag="emt")  # expM^T tiles [k, q]
        rs = [rspool.tile([P, 1], F32, tag=f"rs{h}", name=f"rs{h}") for h in range(2)]

        for h in range(2):
            # logits in a [128, 1024] psum tile (2 banks)
            psl = ps_l.tile([P, 1024], F32, tag="ps_l")
            for n in range(2):
                nc.tensor.matmul(
                    psl[:, n * 512 : (n + 1) * 512],
                    qfT[:, qt * P : (qt + 1) * P],
                    kfT[:, (2 * h + n) * 512 : (2 * h + n + 1) * 512],
                    start=True,
                    stop=True,
                )
            # exp -> bf16 with fp32 row-sum accumulation
            nc.scalar.activation(
                ex[:, h * 1024 : (h + 1) * 1024],
                psl[:],
                AF.Exp,
                scale=float(scale),
                accum_out=rs[h][:],
            )
            # bf16 x bf16 -> bf16 multiply: DVE 2x mode
            nc.vector.tensor_mul(
                em[:, h * 1024 : (h + 1) * 1024],
                ex[:, h * 1024 : (h + 1) * 1024],
                mt[:, h * 1024 : (h + 1) * 1024],
            )

            # previous q-tile's AV runs on PE between this tile's halves
            if pending is not None:
                emit_av(pending)
                pending = None

            # transpose this half's 8 [128,128] blocks; single 2x DVE copy out
            pst = ps_t.tile([P, 8 * P], BF16, tag="ps_t")
            for k in range(8):
                kt = 8 * h + k
                nc.tensor.transpose(
                    pst[:, k * P : (k + 1) * P],
                    em[:, kt * P : (kt + 1) * P],
                    identity_bf[:],
                )
            nc.vector.tensor_copy(
                emt[:, 8 * h : 8 * h + 8, :],
                pst[:].rearrange("p (a b) -> p a b", a=8),
            )

        rsum = rspool.tile([P, 1], F32, tag="rsum")
        nc.vector.tensor_add(rsum[:], rs[0][:], rs[1][:])
        recip = rspool.tile([P, 1], F32, tag="recip", name="recip")
        nc.vector.reciprocal(recip[:], rsum[:])

        pending = (qt, emt, recip)

        # deferred Q-side blocks ride the loop's PE slack (loop is DMA-bound)
        if qt < 3:
            emit_q_block(qt + 1)

    emit_av(pending)


def _build():
    # Bacc (not plain Bass): its compile() legalizes sync waits
    # (move_matmul_waits_to_ldweights + generate_event_semaphores) which
    # walrus codegen requires (max 1 wait per instruction).
    nc = bacc.Bacc("TRN2", target_bir_lowering=False, debug=False)
    io = {}
    for name in ("Q_gene", "K_gene", "Q_expr", "K_expr", "V_expr"):
        io[name] = nc.dram_tensor(name, [G, D], F32, kind="ExternalInput").ap()
    io["M"] = nc.dram_tensor("M", [G, G], F32, kind="ExternalInput").ap()
    for name in ("WK_w", "WQ_w"):
        io[name] = nc.dram_tensor(name, [D, 2 * D], F32, kind="ExternalInput").ap()
    for name in ("WK_b", "WQ_b"):
        io[name] = nc.dram_tensor(name, [D], F32, kind="ExternalInput").ap()
    io["out"] = nc.dram_tensor("out", [G, D], F32, kind="ExternalOutput").ap()

    with tile.TileContext(nc) as tc:
        with ExitStack() as ctx:
            _emit(ctx, tc, io)
    nc.compile()
    return nc


_NC = None


def _get_nc():
    global _NC
    if _NC is None:
        _NC = _build()
    return _NC


def kernel(**inputs) -> np.ndarray:
    return run_kernel_with_results(**inputs)[0]


def run_kernel_with_results(trace=False, **inputs):
    """Returns (full_output, BassKernelResults)."""
    nc = _get_nc()
    per_core_names = ("Q_gene", "K_gene", "Q_expr", "K_expr", "V_expr", "M")
    shared_names = ("WK_w", "WK_b", "WQ_w", "WQ_b")
    arrs = {k: np.ascontiguousarray(np.asarray(v), dtype=np.float32) for k, v in inputs.items()}
    in_maps = []
    for c in range(N_CORES):
        im = {n: arrs[n][c] for n in per_core_names}
        for n in shared_names:
            im[n] = arrs[n]
        in_maps.append(im)
    res = run_bass_kernel_spmd(nc, in_maps, list(range(N_CORES)), trace=trace)
    out = np.stack([res.results[c]["out"] for c in range(N_CORES)], axis=0)
    return out.astype(np.float32), res
